# revision 23
# baseline (speedup 1.0000x reference)
"""GroupSort (k=4) Trainium2 Bass kernel — bf16 planar edition.

x: (16384, 4096) f32. Sort each contiguous group of 4 along the last dim.
Sharding: batch-parallel across 8 NeuronCores (2048 rows/core), no comms.

Numerics: the op is a pure within-group sort; round-to-nearest is
monotone, so sort(round(x)) == round(sort(x)) elementwise. Computing in
bf16 bounds the positionwise relative error by 2^-8 (= 3.9e-3), well
inside the 2e-2 gate, and halves HBM traffic: 16 MiB in + 16 MiB out
per core -> ~94 us at the ~358 GB/s per-NC HBM limit (vs ~187 us f32).

Layout: the host shards each core's rows into 4 de-interleaved planes
(plane j = element j of every group of 4) while converting to bf16, and
re-interleaves the sorted planes on unshard. On device every operand of
the sorting network is then a unit-stride, 4B-aligned bf16 tensor, so
the DVE runs every tensor_tensor in the 2x-packed mode (2 elem/cycle).
DRAM tiles are partition-major ([p, plane, q]) so every DMA is a plain
2D copy with 128 fat contiguous runs.

Batcher network, two comparators per op via dual-plane access patterns
(outer dim = 2, stride 2q over same-tensor plane pairs). SBUF plane
orders are chosen so all six outputs land contiguously:
  tin slot  [P0 P1 P2 P3]
  op1 [A|C]  = min(tin{0,2}, tin{1,3})   -> s1[2q:4q]   (s1 = [B E A C])
  op2 [B|E]  = max(tin{0,2}, tin{1,3})   -> s1[0:2q]
  op3 [M2|O0] = min(s1{0,2}, s1{1,3})    -> s2[0:2q]    (s2 = [M2 O0 O3 M1])
  op4 [O3|M1] = max(s1{0,2}, s1{1,3})    -> s2[2q:4q]
  op5 O1 = min(M1, M2) -> tout[0:q],  op6 O2 = max -> tout[q:2q]
6 TTs/tile, 2.5 element-ops per element, all packed. [O0|O3] is then
contiguous in s2[q:3q] -> one 2-plane store; [O1|O2] is the second.
Output DRAM plane order is [O0 O3 O1 O2]; the host unpack permutes.

Tiles are ramp-shaped [512, 1536, 3072, 4096, 4096, 2048, 1024] (queue-
simulator-optimized): the serial ends (first load before any compute,
last store after all compute, each ~2us sem-receipt) stay small while
mid-stream tiles stay big to amortize per-op overhead.

Raw Bass program (Tile's semaphore pass emits multi-wait DMA
instructions, which the single-wait DIRECT2D ISA struct rejects):
  SP ring:  one whole-tile 2D load per tile (HWDGE)
  DVE:      6 packed min/max ops per tile
  ACT ring: two 2-plane stores per tile (last tile: three, split tail)
"""

import numpy as np
import ml_dtypes

BF16 = np.dtype(ml_dtypes.bfloat16)

B, D, K = 16384, 4096, 4
NCORES = 8
RPC = B // NCORES  # rows per core = 2048
N = RPC * D  # flat elements per core
P = 128  # SBUF partitions
G = D // K  # groups per row = 1024
QS = [512, 1536, 3072, 4096, 4096, 2048, 1024]
QMAX = max(QS)
NT = len(QS)
assert sum(QS) * P * K == N
NBUF = 2

# per-tile DRAM offsets (flat bf16 elements)
OFFS = [0]
for q in QS:
    OFFS.append(OFFS[-1] + K * P * q)

# output plane order in DRAM and its inverse for the host unpack:
# sorted position k is stored in plane OPERM[k]
OPERM = [0, 2, 3, 1]  # planes are [O0 O3 O1 O2]

_cache = {}


def _build():
    import concourse.bass as bass
    import concourse.mybir as mybir

    bf16 = mybir.dt.bfloat16
    mn = mybir.AluOpType.min
    mx = mybir.AluOpType.max

    nc = bass.Bass()
    x = nc.dram_tensor("x", [N], bf16, kind="ExternalInput")
    y = nc.dram_tensor("y", [N], bf16, kind="ExternalOutput")

    def dram2d(h, t):  # [p, 4q] partition-major view of tile t
        return h[OFFS[t] : OFFS[t + 1]].rearrange("(p f) -> p f", p=P)

    # DVE sem increments per tile: after op2/op4/op6 (+op5 on last tile)
    VE_INC = [4 if t == NT - 1 else 3 for t in range(NT)]
    VE_CUM = [0]
    for v in VE_INC:
        VE_CUM.append(VE_CUM[-1] + v)
    with (
        nc.sbuf_tensor([P, NBUF * K * QMAX], bf16) as tin,
        nc.sbuf_tensor([P, 4 * QMAX], bf16) as s1,  # [B E A C]
        nc.sbuf_tensor([P, NBUF * 4 * QMAX], bf16) as s2,  # [M2 O0 O3 M1]
        nc.sbuf_tensor([P, NBUF * 2 * QMAX], bf16) as tout,  # [O1 O2]
        # Parity-split DMA semaphores: each of the 16 SDMA engines incs by
        # 1 per DMA, and engines skew, so an aggregate count over
        # back-to-back DMAs on one sem can be satisfied with a later DMA
        # partially complete. Alternating sems per tile parity keeps at
        # most one in-flight DMA per sem (the consumer wait for tile t-2's
        # DMA precedes tile t's issue), making every count exact.
        nc.semaphore("din0") as din0,
        nc.semaphore("din1") as din1,
        nc.semaphore("ds0") as ds0,  # [O0|O3] stores, even/odd tiles
        nc.semaphore("ds1") as ds1,
        nc.semaphore("dt0") as dt0,  # [O1|O2] stores, even/odd tiles
        nc.semaphore("dt1") as dt1,
        nc.semaphore("ve") as ve,
        nc.Block() as block,
    ):
        DIN = [din0, din1]
        DS = [ds0, ds1]
        DT = [dt0, dt1]

        def dual(buf, base, q, k):
            # [p, 2, q] view of same-tensor planes {k, k+2} out of the 4
            # q-sized planes at buf[:, base:base+4q] (outer stride 2q)
            view = buf[:, base : base + 4 * q]
            return view.rearrange("p (j k q) -> p j k q", j=2, k=2, q=q)[
                :, :, k, :
            ]

        @block.sync
        def _(sync):
            for t in range(NT):
                s = t % NBUF
                if t >= NBUF:
                    # input slot reuse: DVE finished reading tile t-NBUF
                    sync.wait_ge(ve, VE_CUM[t - NBUF] + 1)
                sync.dma_start(
                    tin[:, s * K * QMAX : s * K * QMAX + K * QS[t]],
                    dram2d(x, t),
                ).then_inc(DIN[t % 2], 16)

        @block.vector
        def _(vector):
            for t in range(NT):
                q = QS[t]
                s = t % NBUF
                ti = s * K * QMAX
                so = s * 4 * QMAX
                to = s * 2 * QMAX

                def d3(view):  # [p, 2q] -> [p, 2, q]
                    return view.rearrange("p (j q) -> p j q", j=2)

                in02 = dual(tin, ti, q, 0)  # [P0|P2]
                in13 = dual(tin, ti, q, 1)  # [P1|P3]
                ac = d3(s1[:, 2 * q : 4 * q])  # [A|C]
                be = d3(s1[:, 0 : 2 * q])  # [B|E]
                s1_02 = dual(s1, 0, q, 0)  # [B|A]
                s1_13 = dual(s1, 0, q, 1)  # [E|C]
                m2o0 = d3(s2[:, so : so + 2 * q])
                o3m1 = d3(s2[:, so + 2 * q : so + 4 * q])
                m1 = s2[:, so + 3 * q : so + 4 * q]
                m2 = s2[:, so : so + q]
                o1 = tout[:, to : to + q]
                o2 = tout[:, to + q : to + 2 * q]

                vector.wait_ge(DIN[t % 2], 16 * (t // 2 + 1))
                vector.tensor_tensor(ac, in02, in13, mn)
                vector.tensor_tensor(be, in02, in13, mx)
                # input slot free for the SP ring
                vector.drain().then_inc(ve, 1)
                if t >= NBUF:
                    # s2 slot reuse: [O0|O3] store of tile t-NBUF drained
                    vector.wait_ge(DS[t % 2], 16 * (t // 2))
                vector.tensor_tensor(m2o0, s1_02, s1_13, mn)
                vector.tensor_tensor(o3m1, s1_02, s1_13, mx)
                vector.drain().then_inc(ve, 1)  # [O0|O3] ready
                if t >= NBUF:
                    # tout slot reuse: [O1|O2] store of tile t-NBUF drained
                    vector.wait_ge(DT[t % 2], 16 * (t // 2))
                vector.tensor_tensor(o1, m1, m2, mn)
                if t == NT - 1:
                    vector.drain().then_inc(ve, 1)  # O1 ready (tail split)
                vector.tensor_tensor(o2, m1, m2, mx)
                vector.drain().then_inc(ve, 1)  # O2 (+O1) ready

        @block.scalar
        def _(scalar):
            for t in range(NT):
                q = QS[t]
                s = t % NBUF
                so = s * 4 * QMAX
                to = s * 2 * QMAX
                yv = dram2d(y, t)
                scalar.wait_ge(ve, VE_CUM[t] + 2)
                scalar.dma_start(
                    yv[:, 0 : 2 * q], s2[:, so + q : so + 3 * q]
                ).then_inc(DS[t % 2], 16)
                if t == NT - 1:
                    scalar.wait_ge(ve, VE_CUM[t] + 3)
                    scalar.dma_start(
                        yv[:, 2 * q : 3 * q], tout[:, to : to + q]
                    ).then_inc(DT[t % 2], 16)
                    scalar.wait_ge(ve, VE_CUM[t] + 4)
                    scalar.dma_start(
                        yv[:, 3 * q : 4 * q], tout[:, to + q : to + 2 * q]
                    ).then_inc(DT[(t + 1) % 2], 16)
                else:
                    scalar.wait_ge(ve, VE_CUM[t] + 3)
                    scalar.dma_start(
                        yv[:, 2 * q : 4 * q], tout[:, to : to + 2 * q]
                    ).then_inc(DT[t % 2], 16)

        # Semaphores persist across NEFF executions in this runtime, so a
        # second call would start with every wait pre-satisfied and race.
        # The end-of-block barrier only quiesces engine queues — DMA
        # completion incs can land after it (the final tiles' stores have
        # no consumer wait). The otherwise-idle Pool engine waits (in
        # parallel with the whole run) for every store sem to reach its
        # exact final value — at which point all DMAs have landed and all
        # other engines are past every semaphore use — then clears all
        # sems so re-execution behaves like a first run.
        counts = {id(s): [s, 0] for s in (ds0, ds1, dt0, dt1)}
        for t in range(NT):
            counts[id(DS[t % 2])][1] += 16  # [O0|O3] store
            if t == NT - 1:
                counts[id(DT[t % 2])][1] += 16  # O1 store
                counts[id(DT[(t + 1) % 2])][1] += 16  # O2 store
            else:
                counts[id(DT[t % 2])][1] += 16  # [O1|O2] store
        finals = [(s, v) for s, v in counts.values()]
        all_sems = (din0, din1, ds0, ds1, dt0, dt1, ve)
        nums = sorted(s.num for s in all_sems)
        assert nums == list(range(nums[0], nums[0] + len(nums)))

        @block.gpsimd
        def _(gpsimd):
            for sem, val in finals:
                gpsimd.wait_ge(sem, val)
            gpsimd.sem_clear(range(nums[0], nums[-1] + 1))

    return nc


def _pack(x_np):
    """f32 (B, D) -> per-core planar bf16 flats: concat_t [p, j, q].

    Tile t covers P*q/G rows; partition p holds groups [p*q, (p+1)*q)
    of the tile's row block (groups of 4 never split across partitions).
    """
    xb = np.asarray(x_np, dtype=np.float32).astype(BF16)
    shards = []
    for c in range(NCORES):
        xc = xb[c * RPC : (c + 1) * RPC]
        parts = []
        r0 = 0
        for q in QS:
            rows = P * q // G
            v = xc[r0 : r0 + rows].reshape(P, q, K).transpose(0, 2, 1)
            parts.append(np.ascontiguousarray(v).reshape(-1))
            r0 += rows
        shards.append(np.concatenate(parts))
    return shards


def _unpack(outs):
    """Per-core planar bf16 flats -> f32 (B, D)."""
    y = np.empty((NCORES, RPC, D), dtype=BF16)
    for c in range(NCORES):
        o = outs[c]
        r0 = 0
        for t, q in enumerate(QS):
            rows = P * q // G
            v = o[OFFS[t] : OFFS[t + 1]].reshape(P, K, q)
            v = v[:, OPERM, :]  # planes [O0 O3 O1 O2] -> sorted order
            y[c, r0 : r0 + rows] = v.transpose(0, 2, 1).reshape(rows, D)
            r0 += rows
    return y.reshape(B, D).astype(np.float32)


def _run(x_np, trace=False, trace_kwargs=None):
    from concourse.bass_utils import run_bass_kernel_spmd

    if "nc" not in _cache:
        _cache["nc"] = _build()
    nc = _cache["nc"]

    in_maps = [{"x": s} for s in _pack(x_np)]
    res = run_bass_kernel_spmd(
        nc,
        in_maps,
        list(range(NCORES)),
        trace=trace,
        **(trace_kwargs or {}),
    )
    out = _unpack([np.asarray(r["y"]) for r in res.results])
    return out, res


def kernel(x, k):
    assert int(k) == K, f"kernel hardcodes k={K}, got {k}"
    out, _ = _run(np.asarray(x))
    return out
